# revision 17
# baseline (speedup 1.0000x reference)
"""Multi-head cross-attention (B=8, Nq=1024, Nkv=2048, H=16, D=64) on 8 trn2
NeuronCores, batch-data-parallel (one batch element per core, no collectives).

Host marshaling pre-transposes activations and weights into feature-major
layout, converts to bf16, zeroes masked rows of vision_latents, and ships
(1-mask) broadcast per head; PSUM accumulation is fp32 throughout:
  1. Q^T = Wq^T.T @ X^T; V = VL^T.T @ Wv^T stored ones-column-augmented per
     head [t, 65] where the "ones" column is (1-mask) so masked keys drop out
     of both the numerator and the softmax denominator; K^T = Wk^T.T @ VL^T
     emitted per head-pair chunk, interleaved with attention so the exp
     stream starts as early as possible. All SBUF-resident (no spills).
  2. Per head pair: S^T[t,q] via K=64 row-packed matmul pairs; P^T = exp on
     ACT with scalar scale (no bias AP - that costs ~0.6us/instr on HW);
     A^T accumulated via [t,65] @ P^T so row 64 carries the denominators;
     normalize with a DVE multiply against a gpsimd partition-broadcast
     reciprocal.
  3. O = A_norm^T.T @ Wo^T emitted in natural [n, f] layout, interleaved
     with the second query-chunk's attention.
"""
import numpy as np

B, NQ, NKV = 8, 1024, 2048
QD, KVD, HID = 1024, 1152, 1024
NH, D = 16, 64
NT = NKV // 128          # 16 key tiles
NPAIR = NH // 2          # 8 head pairs
SCALE = 0.125            # 1/sqrt(64)
N_CORES = 8

_cached = {}


def _build_body(nc, tc, io, rep, phases='1234'):
    """Emit one full forward pass. io = dict of dram APs."""
    from concourse import mybir
    from contextlib import ExitStack

    f32 = mybir.dt.float32
    bf16 = mybir.dt.bfloat16
    EXP = mybir.ActivationFunctionType.Exp

    xT_d, vlT_d, nmask_d = io["xT"], io["vlT"], io["nmask_bc"]
    wqT_d, wkT_d, wvT_d, woT_d = io["wqT"], io["wkT"], io["wvT"], io["woT"]
    out_d = io["out"]

    def load_fm(dst_sb, src, C, R, chunk=512):
        """src [C, R] bf16 DRAM (feature-major) -> dst_sb [128, C/128, R].
        rc-outer order so column-block 0 of every contraction slice lands
        first and the first matmul chain can start early."""
        for rc in range(0, R, chunk):
            w = min(chunk, R - rc)
            for ch in range(C // 128):
                nc.sync.dma_start(
                    dst_sb[:, ch, rc:rc + w],
                    src[ch * 128:(ch + 1) * 128, rc:rc + w],
                )

    with ExitStack() as body:
        perm = body.enter_context(tc.tile_pool(name=f"perm{rep}", bufs=1))
        qT_sb = perm.tile([128, QD // 128, NQ], bf16, name=f"qT{rep}")       # Q^T [o, n]
        kT_sb = perm.tile([128, HID // 128, NKV], bf16, name=f"kT{rep}")    # K^T [o, t]
        vaug_sb = perm.tile([128, NT, NH, D + 1], bf16, name=f"vaug{rep}")  # V aug [t, h, d|m]
        woT = perm.tile([128, HID // 128, HID], bf16, name=f"woT{rep}")
        p_pt = body.enter_context(tc.tile_pool(name=f"ph3pt{rep}", bufs=3))
        p_nrm = body.enter_context(tc.tile_pool(name=f"ph3n{rep}", bufs=2))
        p_o = body.enter_context(tc.tile_pool(name=f"ph4o{rep}", bufs=3))

        # unified PSUM plan, all pools open for the whole body (8 banks):
        # pr (2 bufs x 1 bank) for Q/K/V/O chains, s (2 x 2) scores, acc (2 x 1)
        ps_pr = body.enter_context(tc.tile_pool(name=f"pspr{rep}", bufs=2, space="PSUM"))
        ps_s = body.enter_context(tc.tile_pool(name=f"pss{rep}", bufs=2, space="PSUM"))
        ps_a = body.enter_context(tc.tile_pool(name=f"psa{rep}", bufs=1, space="PSUM"))

        _alt = [0]

        def evac(out, in_, ramp):
            # ACT is reserved for exp once attention starts; during ramp
            # alternate ACT/DVE
            _alt[0] ^= 1
            if ramp and _alt[0]:
                nc.scalar.copy(out=out, in_=in_)
            else:
                nc.vector.tensor_copy(out=out, in_=in_)

        def k_proj(oi):
            for tch in range(NKV // 512):
                acc = ps_pr.tile([128, 512], f32, tag="pr", name=f"kpr{rep}_{oi}_{tch}")
                for kj in range(KVD // 128):
                    nc.tensor.matmul(
                        acc[:], wkT[:, kj, oi * 128:(oi + 1) * 128],
                        vlT[:, kj, tch * 512:(tch + 1) * 512],
                        start=(kj == 0), stop=(kj == KVD // 128 - 1),
                    )
                evac(kT_sb[:, oi, tch * 512:(tch + 1) * 512], acc[:], oi == 0)

        def v_proj(och):
            for ti in range(NT):
                acc = ps_pr.tile([128, 512], f32, tag="pr", name=f"vpr{rep}_{ti}_{och}")
                for kj in range(KVD // 128):
                    nc.tensor.matmul(
                        acc[:], vlT[:, kj, ti * 128:(ti + 1) * 128],
                        wvT[:, kj, och * 512:(och + 1) * 512],
                        start=(kj == 0), stop=(kj == KVD // 128 - 1),
                    )
                evac(vaug_sb[:, ti, och * 8:(och + 1) * 8, :D],
                     acc[:].rearrange("t (h d) -> t h d", h=8), och == 0)

        def attention(qch, p):
            accs = [
                ps_a.tile([D + 1, 512], f32, tag=f"acc{h}", name=f"acc{rep}_{p}_{qch}_{h}")
                for h in range(2)
            ]
            for ti in range(NT):
                s_ps = ps_s.tile([128, 2, 512], f32, tag="s", name=f"s{rep}_{p}_{qch}_{ti}")
                for h in range(2):
                    nc.tensor.matmul(
                        s_ps[:, h],
                        kT_sb[h * 64:(h + 1) * 64, p, ti * 128:(ti + 1) * 128],
                        qT_sb[h * 64:(h + 1) * 64, p, qch * 512:(qch + 1) * 512],
                        start=True, stop=True,
                    )
                pT = p_pt.tile([128, 2, 512], bf16, tag="pT", name=f"pT{rep}_{p}_{qch}_{ti}")
                if 'E' in phases:
                    nc.vector.tensor_copy(out=pT[:], in_=s_ps[:])
                else:
                    nc.scalar.activation(
                        pT[:].rearrange("p a b -> p (a b)"),
                        s_ps[:].rearrange("p a b -> p (a b)"),
                        EXP, bias=0.0, scale=SCALE)
                for h in range(2):
                    nc.tensor.matmul(
                        accs[h][:], vaug_sb[:, ti, 2 * p + h], pT[:, h],
                        start=(ti == 0), stop=(ti == NT - 1),
                    )
            for h in range(2):
                rec = p_nrm.tile([1, 512], f32, tag="rec", name=f"rec{rep}_{p}_{qch}_{h}")
                nc.vector.reciprocal(rec[:], accs[h][D:D + 1, :])
                rep_t = p_nrm.tile([64, 512], f32, tag="rep", name=f"rep{rep}_{p}_{qch}_{h}")
                nc.gpsimd.partition_broadcast(rep_t[:], rec[:])
                nc.vector.tensor_tensor(
                    out=anT[h * 64:(h + 1) * 64, p, qch * 512:(qch + 1) * 512],
                    in0=accs[h][:D, :],
                    in1=rep_t[:],
                    op=mybir.AluOpType.mult,
                )

        def o_proj(qt, fch):
            acc = ps_pr.tile([128, 512], f32, tag="pr", name=f"opr{rep}_{qt}_{fch}")
            for oi in range(HID // 128):
                nc.tensor.matmul(
                    acc[:], anT[:, oi, qt * 128:(qt + 1) * 128],
                    woT[:, oi, fch * 512:(fch + 1) * 512],
                    start=(oi == 0), stop=(oi == HID // 128 - 1),
                )
            ost = p_o.tile([128, 512], f32, tag="ost", name=f"ost{rep}_{qt}_{fch}")
            nc.vector.tensor_copy(out=ost[:], in_=acc[:])
            nc.gpsimd.dma_start(
                out_d[qt * 128:(qt + 1) * 128, fch * 512:(fch + 1) * 512], ost[:]
            )

        # ---- phase A: inputs + Q-proj (xT/wqT in a sub-scope, freed after) ----
        pB = body.enter_context(tc.tile_pool(name=f"phB{rep}", bufs=1))
        vlT = pB.tile([128, KVD // 128, NKV], bf16, name=f"vlT{rep}")
        wkT = pB.tile([128, KVD // 128, HID], bf16, name=f"wkT{rep}")
        with ExitStack() as phA:
            p_big = phA.enter_context(tc.tile_pool(name=f"phAbig{rep}", bufs=1))
            xT = p_big.tile([128, QD // 128, NQ], bf16, name=f"xT{rep}")
            wqT = p_big.tile([128, QD // 128, HID], bf16, name=f"wqT{rep}")

            load_fm(wqT, wqT_d, QD, HID)
            load_fm(xT, xT_d, QD, NQ)
            load_fm(vlT, vlT_d, KVD, NKV, chunk=1024)
            load_fm(wkT, wkT_d, KVD, HID)
            # (1-mask) -> denominator column of vaug: contiguous DMA into a
            # staging tile, then one small DVE scatter (a direct strided DMA
            # into column D would be descriptor-bound and block the queue)
            nmask_st = perm.tile([128, NT, NH], bf16, name=f"nmask{rep}")
            nc.sync.dma_start(nmask_st[:], nmask_d[:])
            nc.vector.tensor_copy(out=vaug_sb[:, :, :, D], in_=nmask_st[:])

            for oi in range(HID // 128):
                for nch in range(NQ // 512):
                    acc = ps_pr.tile([128, 512], f32, tag="pr", name=f"qpr{rep}_{oi}_{nch}")
                    for ki in range(QD // 128):
                        nc.tensor.matmul(
                            acc[:], wqT[:, ki, oi * 128:(oi + 1) * 128],
                            xT[:, ki, nch * 512:(nch + 1) * 512],
                            start=(ki == 0), stop=(ki == QD // 128 - 1),
                        )
                    evac(qT_sb[:, oi, nch * 512:(nch + 1) * 512], acc[:], True)

        # ---- phase B/C: wvT + anT in the slot xT/wqT vacated ----
        pC = body.enter_context(tc.tile_pool(name=f"phC{rep}", bufs=1))
        wvT = pC.tile([128, KVD // 128, HID], bf16, name=f"wvT{rep}")
        anT = pC.tile([128, HID // 128, NQ], bf16, name=f"anT{rep}")  # A_norm^T [o, q]
        load_fm(wvT, wvT_d, KVD, HID)
        load_fm(woT, woT_d, HID, HID)  # only needed by O-proj, keep last in queue

        v_proj(0)
        k_proj(0)
        if '3' in phases:
            # qch 0: interleave remaining K-proj chunks and V half 1 between
            # attention pairs so PE feeds ACT without phase barriers
            attention(0, 0)
            k_proj(1)
            attention(0, 1)
            k_proj(2)
            v_proj(1)
            attention(0, 2)
            for p in range(3, NPAIR):
                k_proj(p)
                attention(0, p)
            # qch 1: O-proj of qch 0 rides between pairs
            for p in range(NPAIR):
                attention(1, p)
                if '4' in phases:
                    qt = p // 2
                    o_proj(qt, p % 2)
            if '4' in phases:
                for qt in range(4, 8):
                    for fch in range(HID // 512):
                        o_proj(qt, fch)
        else:
            for p in range(1, NPAIR):
                k_proj(p)
            v_proj(1)


def build_nc(repeat=1, loop_n=0, phases='1234'):
    import concourse.bacc as bacc
    import concourse.tile as tile
    from concourse import mybir

    f32, bf16 = mybir.dt.float32, mybir.dt.bfloat16
    nc = bacc.Bacc("TRN2", target_bir_lowering=False, debug=False)
    io = {
        "xT": nc.dram_tensor("xT", [QD, NQ], bf16, kind="ExternalInput").ap(),
        "vlT": nc.dram_tensor("vlT", [KVD, NKV], bf16, kind="ExternalInput").ap(),
        "nmask_bc": nc.dram_tensor("nmask_bc", [128, NT, NH], bf16, kind="ExternalInput").ap(),
        "wqT": nc.dram_tensor("wqT", [QD, HID], bf16, kind="ExternalInput").ap(),
        "wkT": nc.dram_tensor("wkT", [KVD, HID], bf16, kind="ExternalInput").ap(),
        "wvT": nc.dram_tensor("wvT", [KVD, HID], bf16, kind="ExternalInput").ap(),
        "woT": nc.dram_tensor("woT", [HID, HID], bf16, kind="ExternalInput").ap(),
        "out": nc.dram_tensor("out", [NQ, HID], f32, kind="ExternalOutput").ap(),
    }
    with tile.TileContext(nc) as tc:
        if loop_n:
            with tc.For_i(0, loop_n, 1):
                for rep in range(repeat):
                    rio = dict(io)
                    rio["out"] = nc.dram_tensor(f"scratch_out_{rep}", [NQ, HID], f32).ap()
                    _build_body(nc, tc, rio, rep, phases)
        else:
            for rep in range(repeat):
                rio = dict(io)
                if rep > 0:
                    rio["out"] = nc.dram_tensor(f"scratch_out_{rep}", [NQ, HID], f32).ap()
                _build_body(nc, tc, rio, rep, phases)
    nc.compile()
    return nc


def _bf16(a):
    import ml_dtypes
    return np.ascontiguousarray(np.asarray(a).astype(ml_dtypes.bfloat16))


def _nmask_bc(mask):
    # [NKV] bool/u8 -> [128, NT, NH] bf16 of (1-mask), t = ti*128 + partition
    nm = 1.0 - np.asarray(mask).astype(np.float32)
    return _bf16(np.repeat(nm.reshape(NT, 128).T[:, :, None], NH, axis=2))


def _in_maps(inputs):
    q = np.asarray(inputs["queries"], dtype=np.float32)
    vl = np.asarray(inputs["vision_latents"], dtype=np.float32)
    mask = np.asarray(inputs["attention_mask"])
    wqT = _bf16(np.asarray(inputs["Wq"], dtype=np.float32).T)
    wkT = _bf16(np.asarray(inputs["Wk"], dtype=np.float32).T)
    wvT = _bf16(np.asarray(inputs["Wv"], dtype=np.float32).T)
    woT = _bf16(np.asarray(inputs["Wo"], dtype=np.float32).T)
    m = []
    for c in range(N_CORES):
        nm = 1.0 - mask[c].astype(np.float32)
        m.append({
            "xT": _bf16(q[c].T),
            "vlT": _bf16((vl[c] * nm[:, None]).T),
            "nmask_bc": _nmask_bc(mask[c]),
            "wqT": wqT, "wkT": wkT, "wvT": wvT, "woT": woT,
        })
    return m


def kernel(**inputs) -> np.ndarray:
    from concourse.bass_utils import run_bass_kernel_spmd

    if "nc" not in _cached:
        _cached["nc"] = build_nc(repeat=1)
    nc = _cached["nc"]
    res = run_bass_kernel_spmd(nc, _in_maps(inputs), core_ids=list(range(N_CORES)))
    return np.stack([res.results[c]["out"] for c in range(N_CORES)], axis=0)


if __name__ == "__main__":
    # CoreSim self-check on one core
    from concourse.bass_interp import CoreSim

    nc = build_nc(repeat=1)
    rng = np.random.default_rng(0)
    s = 0.02
    Q = rng.standard_normal((NQ, QD), dtype=np.float32)
    VL = rng.standard_normal((NKV, KVD), dtype=np.float32)
    M = np.zeros(NKV, dtype=np.uint8)
    M[1900:] = 1
    Wq = rng.standard_normal((HID, QD), dtype=np.float32) * s
    Wk = rng.standard_normal((HID, KVD), dtype=np.float32) * s
    Wv = rng.standard_normal((HID, KVD), dtype=np.float32) * s
    Wo = rng.standard_normal((HID, HID), dtype=np.float32) * s

    sim = CoreSim(nc)
    nm = 1.0 - M.astype(np.float32)
    feed = {
        "xT": _bf16(Q.T), "vlT": _bf16((VL * nm[:, None]).T),
        "nmask_bc": _nmask_bc(M),
        "wqT": _bf16(Wq.T), "wkT": _bf16(Wk.T),
        "wvT": _bf16(Wv.T), "woT": _bf16(Wo.T),
    }
    for name, arr in feed.items():
        sim.tensor(name)[:] = arr
    sim.simulate()
    got = np.array(sim.tensor("out"))

    qp = (Q @ Wq.T).reshape(NQ, NH, D).transpose(1, 0, 2)
    kp = (VL @ Wk.T).reshape(NKV, NH, D).transpose(1, 0, 2)
    vp = (VL @ Wv.T).reshape(NKV, NH, D).transpose(1, 0, 2)
    S = np.einsum("hqd,htd->hqt", qp, kp) * SCALE
    S = np.where(M[None, None, :].astype(bool), -1e9, S)
    P = np.exp(S - S.max(-1, keepdims=True))
    P /= P.sum(-1, keepdims=True)
    A = np.einsum("hqt,htd->hqd", P, vp).transpose(1, 0, 2).reshape(NQ, HID)
    want = A @ Wo.T
    rel = np.abs(got - want).max() / np.abs(want).max()
    print("sim rel err:", rel)
    print("sim time (us):", sim.time / 1e3)
